# revision 13
# baseline (speedup 1.0000x reference)
"""Trainium2 Bass kernel for multiresolution hash-grid encoding (Instant-NGP).

Contract: kernel(x01, tables) -> [N, 16] float32, computed on 8 NeuronCores.

Strategy:
  Host: per level, expand the hash table into a dense per-cell "cube grid"
  (row c = the cell's 8 corner embeddings, f16, scaled by 2**16 so raw
  ~1e-4 values sit in f16 normal range). Valid because the hash index of a
  dense grid cell is a constant and masking distributes over XOR. Level 0
  rides in the level-7 row (16 | 256 divisibility): 7 gathers per point.

  Device (data-parallel over points, SPMD on 8 cores): per 128-point
  column j, one SWDGE indirect DMA gathers 128 cube rows (one per
  partition; the HW honors exactly one offset per partition per
  instruction - probed). DVE computes pos = x*R, floor (cast + is_gt
  correction, robust to cast rounding mode), cell index, and a 3-stage
  f16 lerp; the Activation engine does the f32->f16 frac conversion.
  Output accumulates per-batch in a [P, T, 16] f16 tile (levels
  interleaved), one DMA per batch; host unscales to f32.
"""
import math
import numpy as np

NUM_LEVELS = 8
FEATS = 2
TABLE_SIZE = 2 ** 18
MIN_RES = 16
MAX_RES = 256
GROWTH = math.exp(math.log(MAX_RES / MIN_RES) / (NUM_LEVELS - 1))
P1, P2, P3 = 1540863, 1256879, 1957123
RES = [int(math.floor(MIN_RES * GROWTH ** l + 1e-6)) for l in range(NUM_LEVELS)]

N_CORES = 8
P = 128          # SBUF partitions
TP = 2048        # points per partition per core
T_B = 128        # points per partition per batch
PTS_PER_CORE = P * TP          # 262144
N_PAD = N_CORES * PTS_PER_CORE  # 2097152
SCALE = 2.0 ** 16               # f16 normal-range scaling of table values
# levels gathered via the Ant dma_gather ucode (256B super-rows of 8 cube
# rows, int16 super-row indices, DVE one-hot select); () reverts to the
# verified all-indirect path.
DG_LEVELS = ()


def _build_cube_grids(tables: np.ndarray) -> list[np.ndarray]:
    """Per level: [R^3, 16] f16 rows; row = 8 corner embeddings of the cell,
    scaled by SCALE. Level 7 row is 32 f16: [lvl7 cube | lvl0 cube]."""
    grids = [None] * NUM_LEVELS
    mask = TABLE_SIZE - 1
    tabs = (np.asarray(tables, dtype=np.float32) * np.float32(SCALE)).astype(
        np.float16)
    cubes = {}
    for l in range(NUM_LEVELS):
        R = RES[l]
        n = R + 1
        vx = (np.arange(n, dtype=np.int64) * P1)
        vy = (np.arange(n, dtype=np.int64) * P2)
        vz = (np.arange(n, dtype=np.int64) * P3)
        corner_idx = (vx[:, None, None] ^ vy[None, :, None] ^ vz[None, None, :]) & mask
        corner_emb = tabs[l][corner_idx]  # [n, n, n, 2] f16
        cube = np.empty((R, R, R, 8, FEATS), dtype=np.float16)
        e = 0
        for dx in (0, 1):
            for dy in (0, 1):
                for dz in (0, 1):
                    cube[:, :, :, e, :] = corner_emb[dx:dx + R, dy:dy + R, dz:dz + R]
                    e += 1
        cubes[l] = cube.reshape(R ** 3, 8 * FEATS)
        if l not in (0, 7):
            g = cubes[l]
            if l in DG_LEVELS:
                # pad rows to a multiple of 8; view as 256B super-rows
                nr = (R ** 3 + 7) // 8 * 8
                gp = np.zeros((nr, 16), dtype=np.float16)
                gp[:R ** 3] = g
                g = gp.reshape(nr // 8, 128)
            grids[l] = np.ascontiguousarray(g)
    R7, R0 = RES[7], RES[0]
    s = R7 // R0
    c0 = cubes[0].reshape(R0, R0, R0, 16)
    c0e = np.repeat(np.repeat(np.repeat(c0, s, axis=0), s, axis=1), s, axis=2)
    grids[7] = np.ascontiguousarray(
        np.concatenate([cubes[7].reshape(R7, R7, R7, 16), c0e], axis=3)
        .reshape(R7 ** 3, 32)
    )
    return grids


def _host_cells(xc: np.ndarray, R: int) -> np.ndarray:
    """Bit-exact replica of the device cell computation (f32 mult, floor)."""
    pos = xc * np.float32(R)
    i0 = np.floor(pos).astype(np.int32)
    return (i0[:, :, 0] * R + i0[:, :, 1]) * R + i0[:, :, 2]


def _host_dg_inputs(xc: np.ndarray):
    """Per dma_gather level: wrapped int16 super-row indices and f16
    select one-hots for this core's [P, TP, 3] shard."""
    out = {}
    for l in DG_LEVELS:
        cells = _host_cells(xc, RES[l])            # [P, TP] int32
        sup = (cells >> 3).astype(np.int16)
        sel = (cells & 7).astype(np.int64)
        oh = np.zeros((P, TP, 8), dtype=np.float16)
        np.put_along_axis(oh, sel[:, :, None], np.float16(1.0), axis=2)
        # batch b, point (p, t): list position k = p + 128*t ->
        # idxw[k % 16, k // 16]; replicate the 16-row block to 128
        nb = TP // T_B
        idxw = np.zeros((16, nb * (T_B * 128 // 16)), dtype=np.int16)
        w = T_B * 128 // 16                        # words per batch
        for b in range(nb):
            blk = sup[:, b * T_B:(b + 1) * T_B]    # [P, T]
            k = (np.arange(P)[:, None] + 128 * np.arange(T_B)[None, :])
            idxw[:, b * w:(b + 1) * w].reshape(16, w)[
                k.ravel() % 16, k.ravel() // 16] = blk.ravel()
        out[f"ix{l}"] = np.ascontiguousarray(np.tile(idxw, (8, 1)))
        out[f"oh{l}"] = oh
    return out


def _build_program():
    import concourse.bass as bass
    import concourse.bacc as bacc
    import concourse.tile as tile
    from concourse import mybir

    f32 = mybir.dt.float32
    f16 = mybir.dt.float16
    i32 = mybir.dt.int32
    Alu = mybir.AluOpType
    Act = mybir.ActivationFunctionType

    nc = bacc.Bacc("TRN2", target_bir_lowering=False, debug=False)
    x_ext = nc.dram_tensor("x", [P, TP, 3], f32, kind="ExternalInput")
    g_ext = {
        l: nc.dram_tensor(f"g{l}", [RES[l] ** 3, 16], f16, kind="ExternalInput")
        for l in range(1, NUM_LEVELS - 1)
    }
    g_ext[7] = nc.dram_tensor("g7", [RES[7] ** 3, 32], f16, kind="ExternalInput")
    out_ext = nc.dram_tensor("out", [P, TP, NUM_LEVELS * FEATS], f16,
                             kind="ExternalOutput")

    n_batches = TP // T_B
    T = T_B
    LVLS = (7, 0, 1, 2, 3, 4, 5, 6)

    with tile.TileContext(nc) as tc:
        with (
            tc.tile_pool(name="xp", bufs=2) as xp,
            tc.tile_pool(name="coord", bufs=16) as coord,
            tc.tile_pool(name="idxp", bufs=14) as idxp,
            tc.tile_pool(name="cubep", bufs=12) as cubep,
            tc.tile_pool(name="lerpp", bufs=2) as lerpp,
            tc.tile_pool(name="outp", bufs=2) as outp,
        ):
            tiles = {}   # (b, l) -> (idx | None, fh)
            cubes = {}   # (b, l) -> cube tile

            def emit_coords(b):
                """xt load + per-level floor/frac/cell/idx; DVE+act only."""
                xt = xp.tile([P, T * 3], f32, tag="x")
                nc.sync.dma_start(out=xt[:],
                                  in_=x_ext.ap()[:, b * T:(b + 1) * T, :])
                for l in LVLS:
                    R = RES[l]
                    pos = coord.tile([P, T * 3], f32, tag="pos", bufs=2)
                    nc.vector.tensor_scalar_mul(pos[:], xt[:], float(R))
                    # floor robust to either cast rounding mode
                    icst = coord.tile([P, T * 3], i32, tag="icst", bufs=2)
                    nc.vector.tensor_copy(out=icst[:], in_=pos[:])
                    i0f = coord.tile([P, T * 3], f32, tag="i0f", bufs=2)
                    nc.vector.tensor_copy(out=i0f[:], in_=icst[:])
                    up = coord.tile([P, T * 3], f32, tag="up", bufs=2)
                    nc.vector.tensor_tensor(
                        out=up[:], in0=i0f[:], in1=pos[:], op=Alu.is_gt)
                    nc.vector.tensor_tensor(
                        out=i0f[:], in0=i0f[:], in1=up[:], op=Alu.subtract)
                    frac = coord.tile([P, T * 3], f32, tag="frac", bufs=2)
                    nc.vector.tensor_tensor(
                        out=frac[:], in0=pos[:], in1=i0f[:], op=Alu.subtract)
                    fh = coord.tile([P, T * 3], f16, tag="fh", bufs=16)
                    nc.scalar.activation(fh[:], frac[:], Act.Copy,
                                         bias=0.0, scale=1.0)
                    idx = None
                    if l != 0:
                        i3 = i0f[:].rearrange("p (t c) -> p t c", c=3)
                        ix, iy, iz = i3[:, :, 0], i3[:, :, 1], i3[:, :, 2]
                        cell = coord.tile([P, T], f32, tag="cell", bufs=2)
                        nc.vector.scalar_tensor_tensor(
                            out=cell[:], in0=ix, scalar=float(R), in1=iy,
                            op0=Alu.mult, op1=Alu.add)
                        nc.vector.scalar_tensor_tensor(
                            out=cell[:], in0=cell[:], scalar=float(R), in1=iz,
                            op0=Alu.mult, op1=Alu.add)
                        idx = idxp.tile([P, T], i32, tag="idx", bufs=14)
                        nc.vector.tensor_copy(out=idx[:], in_=cell[:])
                    tiles[(b, l)] = (idx, fh)

            def emit_gathers(b):
                """7 groups of 128 single-offset indirect DMAs (Pool)."""
                for l in LVLS:
                    if l == 0:
                        cubes[(b, 0)] = cubes[(b, 7)]
                        continue
                    rowf = 32 if l == 7 else 16
                    if l == 7:
                        cube = cubep.tile([P, T * 32], f16, tag="cube7",
                                          bufs=3)
                    else:
                        cube = cubep.tile([P, T * 16], f16, tag="cube",
                                          bufs=12)
                    idx = tiles[(b, l)][0]
                    for j in range(T):
                        nc.gpsimd.indirect_dma_start(
                            out=cube[:, j * rowf:(j + 1) * rowf],
                            out_offset=None,
                            in_=g_ext[l].ap(),
                            in_offset=bass.IndirectOffsetOnAxis(
                                ap=idx[:, j:j + 1], axis=0),
                        )
                    cubes[(b, l)] = cube

            def emit_lerps(b):
                obuf = outp.tile([P, T * 16], f16, tag="obuf")
                o4 = obuf[:].rearrange("p (t l f) -> p t l f", l=8, f=2)
                for l in LVLS:
                    cube = cubes[(b, l)]
                    fh = tiles[(b, l)][1]
                    f3 = fh[:].rearrange("p (t c) -> p t c", c=3)
                    fx, fy, fz = f3[:, :, 0], f3[:, :, 1], f3[:, :, 2]
                    if l in (7, 0):
                        c6 = cube[:].rearrange(
                            "p (t h a z f) -> p t h a z f", h=2, a=4, z=2, f=2)
                        h = 0 if l == 7 else 1
                        e0, e1 = c6[:, :, h, :, 0, :], c6[:, :, h, :, 1, :]
                    else:
                        cz = cube[:].rearrange(
                            "p (t a z f) -> p t a z f", a=4, z=2, f=2)
                        e0, e1 = cz[:, :, :, 0, :], cz[:, :, :, 1, :]
                    az = lerpp.tile([P, T * 8], f16, tag="az")
                    az4 = az[:].rearrange("p (t a f) -> p t a f", a=4, f=2)
                    dz = lerpp.tile([P, T * 8], f16, tag="dz")
                    dz4 = dz[:].rearrange("p (t a f) -> p t a f", a=4, f=2)
                    nc.vector.tensor_tensor(out=dz4, in0=e1, in1=e0,
                                            op=Alu.subtract)
                    nc.vector.tensor_tensor(
                        out=dz4, in0=dz4, in1=fz.to_broadcast([P, T, 4, 2]),
                        op=Alu.mult)
                    nc.vector.tensor_tensor(out=az4, in0=dz4, in1=e0,
                                            op=Alu.add)
                    ay = lerpp.tile([P, T * 4], f16, tag="ay")
                    ay4 = ay[:].rearrange("p (t a f) -> p t a f", a=2, f=2)
                    azy = az[:].rearrange(
                        "p (t a y f) -> p t a y f", a=2, y=2, f=2)
                    y0, y1 = azy[:, :, :, 0, :], azy[:, :, :, 1, :]
                    dy_ = lerpp.tile([P, T * 4], f16, tag="dy")
                    dy4 = dy_[:].rearrange("p (t a f) -> p t a f", a=2, f=2)
                    nc.vector.tensor_tensor(out=dy4, in0=y1, in1=y0,
                                            op=Alu.subtract)
                    nc.vector.tensor_tensor(
                        out=dy4, in0=dy4, in1=fy.to_broadcast([P, T, 2, 2]),
                        op=Alu.mult)
                    nc.vector.tensor_tensor(out=ay4, in0=dy4, in1=y0,
                                            op=Alu.add)
                    ol2 = o4[:, :, l, :]
                    ayx = ay[:].rearrange("p (t x f) -> p t x f", x=2, f=2)
                    x0, x1 = ayx[:, :, 0, :], ayx[:, :, 1, :]
                    dx_ = lerpp.tile([P, T * 2], f16, tag="dx")
                    dx2 = dx_[:].rearrange("p (t f) -> p t f", f=2)
                    nc.vector.tensor_tensor(out=dx2, in0=x1, in1=x0,
                                            op=Alu.subtract)
                    nc.vector.tensor_tensor(
                        out=dx2, in0=dx2, in1=fx.to_broadcast([P, T, 2]),
                        op=Alu.mult)
                    nc.vector.tensor_tensor(out=ol2, in0=dx2, in1=x0,
                                            op=Alu.add)
                nc.sync.dma_start(
                    out=out_ext.ap()[:, b * T:(b + 1) * T, :],
                    in_=obuf[:])

            # software pipeline: coords run one batch ahead of lerps so the
            # Pool gather stream never waits on DVE index computation.
            emit_coords(0)
            for b in range(n_batches):
                emit_gathers(b)
                if b + 1 < n_batches:
                    emit_coords(b + 1)
                emit_lerps(b)
    _strip_redundant_gather_waits(nc)
    nc.compile()
    return nc


def _strip_redundant_gather_waits(nc):
    """Within each run of consecutive qPoolDynamic gathers that share the
    same sync dependency (the idx-write), keep the semaphore wait on the
    first instruction only; the Pool engine is serial, so later gathers in
    the run are ordered behind it anyway. Demote their deps to nosync.
    Cuts ~14k SEQ wait-processing stalls (~300ns each)."""
    from concourse.tile import InstructionNameOrderedSet
    try:
        fn = nc.m.functions[0]
        for blk in fn.blocks:
            prev_sync = None
            for inst in blk.instructions:
                is_gather = (type(inst).__name__ == "InstDMACopy"
                             and getattr(inst, "queue", "") == "qPoolDynamic")
                if not is_gather:
                    prev_sync = None
                    continue
                cur = tuple(sorted(inst.sync_dependency_names()))
                if cur and cur == prev_sync:
                    merged = sorted(set(inst.nosync_dependency_names())
                                    | set(cur))
                    inst.set_sync_dependencies(InstructionNameOrderedSet([]))
                    inst.set_nosync_dependencies(
                        InstructionNameOrderedSet(merged))
                else:
                    prev_sync = cur
    except Exception:
        pass  # fall back to the fully-synced program


_PROGRAM_CACHE = {}


def kernel(x01: np.ndarray, tables: np.ndarray, _trace: bool = False,
           _tmpdir: str | None = None) -> np.ndarray:
    from concourse.bass_utils import run_bass_kernel_spmd

    N = x01.shape[0]
    assert N <= N_PAD, (N, N_PAD)

    grids = _build_cube_grids(tables)

    xp = np.zeros((N_PAD, 3), dtype=np.float32)
    xp[:N] = np.asarray(x01, dtype=np.float32)
    # keep pos strictly below R so cell stays in-bounds
    np.clip(xp, 0.0, np.float32(1.0 - 2 ** -24), out=xp)

    key = "prog"
    if key not in _PROGRAM_CACHE:
        _PROGRAM_CACHE[key] = _build_program()
    nc = _PROGRAM_CACHE[key]

    in_maps = []
    for c in range(N_CORES):
        xc = xp[c * PTS_PER_CORE:(c + 1) * PTS_PER_CORE].reshape(P, TP, 3)
        m = {"x": xc}
        for l in range(1, NUM_LEVELS):
            m[f"g{l}"] = grids[l]
        m.update(_host_dg_inputs(xc))
        in_maps.append(m)

    res = run_bass_kernel_spmd(
        nc, in_maps, core_ids=list(range(N_CORES)),
        trace=_trace, tmpdir=_tmpdir,
    )

    # out per core: [P, TP, 16] f16 (scaled) -> [N, 16] f32
    parts = [r["out"].reshape(PTS_PER_CORE, NUM_LEVELS * FEATS)
             for r in res.results]
    out = np.concatenate(parts, axis=0).astype(np.float32) * np.float32(1.0 / SCALE)
    if _trace:
        kernel.last_exec_time_ns = res.exec_time_ns
        kernel.last_results = res
    return np.ascontiguousarray(out[:N])
